# revision 16
# baseline (speedup 1.0000x reference)
"""DeformConv1d v4: block-diag K=128 matmuls, big-descriptor broadcast,
half-L pipelining.  See kernel_v3.py docstring for the math; changes here:

- Matmuls contract K=128 (both batches) with block-diagonal weights
  [[W,0],[0,W]] -> full PE array, FWL weight loads, no tile_position.
- mw_dram is chunk-major [b, ci, f, LC]; a windowed staging tile holds 8
  port-spread replicas per batch of the current chunk's 6 arrays; the
  64-way broadcast runs as 16 dma_starts x 8 descriptors x 12KB.
- relu(-s) on Vector tensor_scalar (4x mode), relu(s) on Scalar ACT.
- Phase-1 conv of the second L-half rides inside the first half's
  product loop (PE is idle there); smallside runs per half.
"""

import numpy as np
import ml_dtypes

import concourse.bacc as bacc
import concourse.mybir as mybir
from concourse.ap import AP
from concourse.tile import TileContext
from concourse.bass_utils import run_bass_kernel_spmd

B, C, CO, L, K = 16, 64, 64, 16384, 3
NCORES = 8
BLOC = B // NCORES
HALO = 8
LP = L + 2 * HALO
LC = 1024
NCH = L // LC
MMN = 512
DD = LC + 6
SIXLC = 6 * LC
BF = mybir.dt.bfloat16
F32 = mybir.dt.float32
BF_NP = ml_dtypes.bfloat16

_BUILD_CACHE = {}

KS = [0, 1, 2, 0, 1, 2, 1, 0, 2]
SGN = [1, 1, 1, -1, -1, -1, 1, 1, 1]


def _pv(base, off0, off1, n):
    return AP(tensor=base.tensor, offset=base.offset + off0,
              ap=[[base.ap[0][0], base.ap[0][1]], [off1 - off0, 2], [1, n]])


def _build():
    if "nc" in _BUILD_CACHE:
        return _BUILD_CACHE["nc"]
    nc = bacc.Bacc("TRN2")

    x2r = nc.dram_tensor("x2r", [128, LP], BF, kind="ExternalInput")
    x2ro = nc.dram_tensor("x2ro", [128, LP], BF, kind="ExternalInput")
    wbr = nc.dram_tensor("wbr", [128, 7 * 12], BF, kind="ExternalInput")
    bbr = nc.dram_tensor("bbr", [128, 12], BF, kind="ExternalInput")
    wvr = nc.dram_tensor("wvr", [128, 9 * 128], BF, kind="ExternalInput")
    bvo = nc.dram_tensor("bvo", [128, 128], BF, kind="ExternalInput")
    out_t = nc.dram_tensor("out", [128, L], BF, kind="ExternalOutput")

    br_dram = nc.dram_tensor("br_scratch", [BLOC, 6, L], BF)
    mw_dram = nc.dram_tensor("mw_scratch", [BLOC, 6, L], BF)

    H = HALO
    with TileContext(nc) as tc:
        with (
            tc.tile_pool(name="big", bufs=1) as bigp,
            tc.tile_pool(name="const", bufs=1) as constp,
            tc.tile_pool(name="p1", bufs=2) as p1p,
            tc.tile_pool(name="seg", bufs=1) as segp,
            tc.tile_pool(name="mwp", bufs=3) as mwp,
            tc.tile_pool(name="rrp", bufs=2) as rrp,
            tc.tile_pool(name="valp", bufs=2) as valp,
            tc.tile_pool(name="ddp", bufs=2) as ddp,
            tc.tile_pool(name="outp", bufs=2) as outp,
            tc.tile_pool(name="ps1", bufs=2, space="PSUM") as ps1p,
            tc.tile_pool(name="pso", bufs=2, space="PSUM") as psop,
        ):
            xx = bigp.tile([128, 2 * LP], BF, tag="xx")
            XO, XR = 0, LP
            # x2r half chunked so phase-1 can start early
            for c4 in range(4):
                c0 = c4 * (LP // 4)
                nc.sync.dma_start(out=xx[:, XR + c0:XR + c0 + LP // 4],
                                  in_=x2r[:, c0:c0 + LP // 4])
            for c2 in range(2):
                c0 = c2 * (LP // 2)
                nc.scalar.dma_start(out=xx[:, XO + c0:XO + c0 + LP // 2],
                                    in_=x2ro[:, c0:c0 + LP // 2])
            wbr_sb = constp.tile([128, 7 * 12], BF, tag="wbr")
            nc.sync.dma_start(out=wbr_sb[:], in_=wbr[:])
            bbr_sb = constp.tile([128, 12], BF, tag="bbr")
            nc.sync.dma_start(out=bbr_sb[:], in_=bbr[:])
            wvr_sb = constp.tile([128, 9 * 128], BF, tag="wvr")
            nc.sync.dma_start(out=wvr_sb[:], in_=wvr[:])
            bvo_sb = constp.tile([128, 128], BF, tag="bvo")
            nc.sync.dma_start(out=bvo_sb[:], in_=bvo[:])
            ones = constp.tile([128, MMN], BF, tag="ones")
            nc.vector.memset(ones[:], 1.0)
            brseg = segp.tile([128, BLOC * 6 * 128], BF, tag="brseg")
            mwseg = segp.tile([128, BLOC * 6 * 128], BF, tag="mwseg")

            def phase1_chunk(i):
                # h0 chain on PE col-group 0, h1 on col-group 32: the two
                # 8-pass accumulation chains run concurrently on the PE
                ps = ps1p.tile([44, LC], F32, tag="brps", name="brps")
                for tau in range(7):
                    for h in range(2):
                        o0 = XR + H + i * LC + h * MMN
                        r0 = 32 * h
                        nc.tensor.matmul(
                            ps[r0:r0 + 12, h * MMN:(h + 1) * MMN],
                            lhsT=wbr_sb[:, tau * 12:(tau + 1) * 12],
                            rhs=xx[:, o0 + tau - 3:o0 + tau - 3 + MMN],
                            start=(tau == 0), stop=False,
                            tile_position=(0, r0),
                        )
                for h in range(2):
                    r0 = 32 * h
                    nc.tensor.matmul(
                        ps[r0:r0 + 12, h * MMN:(h + 1) * MMN],
                        lhsT=bbr_sb[0:1, :], rhs=ones[0:1, :],
                        start=False, stop=True, tile_position=(0, r0),
                    )
                brst = p1p.tile([44, LC], BF, tag="brst", name="brst")
                nc.vector.tensor_copy(out=brst[0:12, 0:MMN], in_=ps[0:12, 0:MMN])
                nc.scalar.activation(
                    out=brst[32:44, MMN:LC], in_=ps[32:44, MMN:LC],
                    func=mybir.ActivationFunctionType.Identity)
                for b in range(2):
                    for h in range(2):
                        eng = nc.sync if (b + h) % 2 == 0 else nc.scalar
                        eng.dma_start(
                            out=br_dram[b, :, i * LC + h * MMN:
                                        i * LC + (h + 1) * MMN],
                            in_=brst[32 * h + 6 * b:32 * h + 6 * b + 6,
                                     h * MMN:(h + 1) * MMN])

            def smallside(hf):
                rows = slice(64 * hf, 64 * hf + 64)
                l0, l1 = hf * (L // 2), (hf + 1) * (L // 2)
                nc.sync.dma_start(
                    out=brseg[rows].rearrange("p (b f j) -> p b f j", b=BLOC, f=6),
                    in_=br_dram[:, :, l0:l1].rearrange("b f (p j) -> p b f j", j=128),
                )
                brv = brseg[rows].rearrange("p (b f j) -> p b f j", b=BLOC, f=6)
                mwv = mwseg[rows].rearrange("p (b f j) -> p b f j", b=BLOC, f=6)
                for b in range(BLOC):
                    nc.scalar.activation(
                        out=mwv[:, b, 0:3, :], in_=brv[:, b, 3:6, :],
                        func=mybir.ActivationFunctionType.Sigmoid)
                for b in range(BLOC):
                    nc.vector.tensor_tensor(
                        out=mwv[:, b, 3:6, :], in0=mwv[:, b, 0:3, :],
                        in1=brv[:, b, 0:3, :], op=mybir.AluOpType.mult)
                nc.scalar.dma_start(
                    out=mw_dram[:, :, l0:l1].rearrange(
                        "b f (p j) -> p b f j", j=128),
                    in_=mwseg[rows].rearrange(
                        "p (b f j) -> p b f j", b=BLOC, f=6),
                )

            def phase2_chunk(ci):
                l0 = ci * LC
                # 64-way broadcast straight from DRAM (HBM bw, not SBUF ports):
                # one dma per batch, 64 descriptors x 12KB each
                mw2 = mwp.tile([128, SIXLC], BF, tag="mw2", name="mw2")
                for b in range(2):
                    eng = nc.sync if b == 0 else nc.scalar
                    eng.dma_start(
                        out=mw2[64 * b:64 * b + 64, :].rearrange(
                            "p (f l) -> p f l", f=6),
                        in_=mw_dram[b, :, l0:l0 + LC].unsqueeze(0)
                            .partition_broadcast(64))

                dd = ddp.tile([128, 2 * DD], BF, tag="dd", name="dd")
                nc.gpsimd.tensor_tensor(
                    out=dd[:, 0:LC + 4],
                    in0=xx[:, XO + H + l0 - 2:XO + H + l0 + LC + 2],
                    in1=xx[:, XR + H + l0 - 2:XR + H + l0 + LC + 2],
                    op=mybir.AluOpType.subtract)
                nc.gpsimd.tensor_tensor(
                    out=dd[:, DD:DD + LC + 2],
                    in0=xx[:, XR + H + l0:XR + H + l0 + LC + 2],
                    in1=xx[:, XO + H + l0 - 2:XO + H + l0 + LC],
                    op=mybir.AluOpType.subtract)

                rr = rrp.tile([128, SIXLC], BF, tag="rr", name="rr")
                # rm = relu(-s): alternate V tensor_scalar / S ACT by parity
                if ci % 2 == 0:
                    nc.vector.tensor_scalar(
                        out=rr[:, 0:3 * LC], in0=mw2[:, 3 * LC:6 * LC],
                        scalar1=-1.0, scalar2=0.0,
                        op0=mybir.AluOpType.mult, op1=mybir.AluOpType.max)
                else:
                    nc.scalar.activation(
                        out=rr[:, 0:3 * LC], in_=mw2[:, 3 * LC:6 * LC],
                        func=mybir.ActivationFunctionType.Relu, scale=-1.0)
                # rp: [rp1, rp0, rp2] slots; on S
                nc.scalar.activation(
                    out=rr[:, 3 * LC:4 * LC], in_=mw2[:, 4 * LC:5 * LC],
                    func=mybir.ActivationFunctionType.Relu)
                nc.scalar.activation(
                    out=rr[:, 4 * LC:6 * LC].rearrange("p (q j) -> p q j", q=2),
                    in_=_pv(mw2[:], 3 * LC, 5 * LC, LC),
                    func=mybir.ActivationFunctionType.Relu)

                val = valp.tile([128, 9 * LC], BF, tag="val", name="val")
                nc.vector.tensor_tensor(
                    out=val[:, 0:2 * LC].rearrange("p (q j) -> p q j", q=2),
                    in0=mw2[:, 0:2 * LC].rearrange("p (q j) -> p q j", q=2),
                    in1=_pv(xx[:], XO + H + l0 - 2, XR + H + l0, LC),
                    op=mybir.AluOpType.mult)
                nc.vector.tensor_tensor(
                    out=val[:, 2 * LC:3 * LC], in0=mw2[:, 2 * LC:3 * LC],
                    in1=xx[:, XO + H + l0:XO + H + l0 + LC],
                    op=mybir.AluOpType.mult)
                de0, de2 = 0, 2
                do0, do2 = DD, DD + 2
                nc.vector.tensor_tensor(
                    out=val[:, 3 * LC:5 * LC].rearrange("p (q j) -> p q j", q=2),
                    in0=rr[:, 0:2 * LC].rearrange("p (q j) -> p q j", q=2),
                    in1=_pv(dd[:], de0, do0, LC),
                    op=mybir.AluOpType.mult)
                nc.vector.tensor_tensor(
                    out=_pv(val[:], 5 * LC, 8 * LC, LC),
                    in0=_pv(rr[:], 2 * LC, 5 * LC, LC),
                    in1=_pv(dd[:], de2, do2, LC),
                    op=mybir.AluOpType.mult)
                nc.vector.tensor_tensor(
                    out=val[:, 6 * LC:8 * LC].rearrange("p (q j) -> p q j", q=2),
                    in0=rr[:, 3 * LC:5 * LC].rearrange("p (q j) -> p q j", q=2),
                    in1=_pv(dd[:], de2, do0, LC),
                    op=mybir.AluOpType.mult)

                osb = outp.tile([128, LC], BF, tag="osb", name="osb")
                psos = [psop.tile([128, MMN], F32, tag=f"pso{nh}", name=f"pso{nh}")
                        for nh in range(2)]
                # a-outer, col-split M=64 (2 col-tiles run concurrently),
                # nh-inner reuses each weight tile twice
                for a in range(9):
                    for cg in range(2):
                        for nh in range(2):
                            nc.tensor.matmul(
                                psos[nh][64 * cg:64 * cg + 64, :],
                                lhsT=wvr_sb[:, a * 128 + 64 * cg:
                                            a * 128 + 64 * cg + 64],
                                rhs=val[:, a * LC + nh * MMN:
                                        a * LC + (nh + 1) * MMN],
                                start=(a == 0), stop=False,
                                tile_position=(0, 64 * cg),
                            )
                for cg in range(2):
                    for nh in range(2):
                        nc.tensor.matmul(
                            psos[nh][64 * cg:64 * cg + 64, :],
                            lhsT=bvo_sb[0:1, 64 * cg:64 * cg + 64],
                            rhs=ones[0:1, :], start=False, stop=True,
                            tile_position=(0, 64 * cg),
                        )
                for nh in range(2):
                    nc.scalar.activation(
                        out=osb[:, nh * MMN:(nh + 1) * MMN], in_=psos[nh][:],
                        func=mybir.ActivationFunctionType.Identity)
                eng = nc.sync if ci % 2 == 0 else nc.scalar
                eng.dma_start(out=out_t[:, l0:l0 + LC], in_=osb[:])

            for i in range(8):
                phase1_chunk(i)
            smallside(0)
            for ci in range(8):
                phase2_chunk(ci)
                # front-load the H1 branch convs (2 per chunk) so
                # smallside(1) overlaps the tail of H0's products
                if ci < 4:
                    phase1_chunk(8 + 2 * ci)
                    phase1_chunk(9 + 2 * ci)
                if ci == 4:
                    smallside(1)
            for ci in range(8, 16):
                phase2_chunk(ci)

    nc.compile()
    _BUILD_CACHE["nc"] = nc
    return nc


def _host_prep(inputs):
    x = np.asarray(inputs["x"], np.float32)
    w_off_dw = np.asarray(inputs["w_off_dw"], np.float32)
    b_off_dw = np.asarray(inputs["b_off_dw"], np.float32)
    w_off_pw = np.asarray(inputs["w_off_pw"], np.float32)
    b_off_pw = np.asarray(inputs["b_off_pw"], np.float32)
    w_msk_dw = np.asarray(inputs["w_msk_dw"], np.float32)
    b_msk_dw = np.asarray(inputs["b_msk_dw"], np.float32)
    w_msk_pw = np.asarray(inputs["w_msk_pw"], np.float32)
    b_msk_pw = np.asarray(inputs["b_msk_pw"], np.float32)
    weight = np.asarray(inputs["weight"], np.float32)
    bias = np.asarray(inputs["bias"], np.float32)

    wf = np.zeros((C, 7, 6), np.float32)
    wf[:, :, 0:3] = (w_off_pw[:, :, 0].T[:, None, :] * w_off_dw[:, 0, :][:, :, None])
    wf[:, :, 3:6] = (w_msk_pw[:, :, 0].T[:, None, :] * w_msk_dw[:, 0, :][:, :, None])
    beff = np.zeros(6, np.float32)
    beff[0:3] = b_off_pw + w_off_pw[:, :, 0] @ b_off_dw
    beff[3:6] = b_msk_pw + w_msk_pw[:, :, 0] @ b_msk_dw

    # block-diag branch weights: [128, 7*12], col tau*12 + m
    wbr_h = np.zeros((128, 7 * 12), np.float32)
    for tau in range(7):
        wbr_h[0:64, tau * 12 + 0:tau * 12 + 6] = wf[:, tau, :]
        wbr_h[64:128, tau * 12 + 6:tau * 12 + 12] = wf[:, tau, :]
    wbr_h = wbr_h.astype(BF_NP)
    bbr_h = np.zeros((128, 12), np.float32)
    bbr_h[:, 0:6] = beff
    bbr_h[:, 6:12] = beff
    bbr_h = bbr_h.astype(BF_NP)
    # block-diag val weights: [128, 9*128]
    wvr_h = np.zeros((128, 9 * 128), np.float32)
    for a in range(9):
        wa = SGN[a] * weight[:, :, KS[a]].T          # [c, o]
        wvr_h[0:64, a * 128 + 0:a * 128 + 64] = wa
        wvr_h[64:128, a * 128 + 64:a * 128 + 128] = wa
    wvr_h = wvr_h.astype(BF_NP)
    bvo_h = np.zeros((128, 128), np.float32)
    bvo_h[:, 0:64] = bias
    bvo_h[:, 64:128] = bias
    bvo_h = bvo_h.astype(BF_NP)

    in_maps = []
    for core in range(NCORES):
        xb = x[core * BLOC:(core + 1) * BLOC].astype(BF_NP)
        x2r_h = np.zeros((128, LP), BF_NP)
        x2r_h[:, HALO:HALO + L] = xb.reshape(128, L)
        x2ro_h = np.zeros_like(x2r_h)
        x2ro_h[:, :LP - 1] = x2r_h[:, 1:]
        in_maps.append({
            "x2r": x2r_h, "x2ro": x2ro_h, "wbr": wbr_h, "bbr": bbr_h,
            "wvr": wvr_h, "bvo": bvo_h,
        })
    return in_maps


def kernel(**inputs):
    nc = _build()
    in_maps = _host_prep(inputs)
    res = run_bass_kernel_spmd(nc, in_maps, list(range(NCORES)))
    out = np.empty((B, CO, L), np.float32)
    for core in range(NCORES):
        out[core * BLOC:(core + 1) * BLOC] = (
            res.results[core]["out"].astype(np.float32).reshape(BLOC, CO, L))
    return out
